# revision 1
# baseline (speedup 1.0000x reference)
"""Single attention head (B=8, S=2048, D_IN=1024, D_OUT=64) on 8 TRN2 NeuronCores.

Strategy: pure data-parallel over batch -- core b computes batch element b's
full attention head. No collectives.

Per-core dataflow (all matmul compute in bf16, f32 PSUM accumulation):
  phase 1: qT/kT/vT = W.T @ seqT  ([64, 2048], f on partitions), bias added
           during the PSUM->SBUF drain on VectorE.  vT is re-transposed on the
           TensorEngine into natural [k, 65] layout with an appended
           ones-column (so the context matmul also produces softmax
           denominators for free).
  phase 2: for each key-chunk [128 keys]:
             scoresT[k, q] = kT_chunk.T @ qT            (PE, K=64)
             e = exp(scores/sqrt(S) + mask_bias[k])     (ScalarE; mask rides
                 the per-partition bias port; no max-subtraction needed since
                 |scores/sqrt(S)| << 1 and masked lanes underflow to 0)
             ctxT[0:64, q] += v_chunk.T @ e; ctxT[64, q] += sum_k e  (PE)
  phase 3: transpose ctxT back to [q, 65], multiply rows by 1/ctx[.., 64],
           DMA the [2048, 64] f32 result out.
"""

import numpy as np
import ml_dtypes

import concourse.bass as bass  # noqa: F401  (bass types used via tile/bacc)
import concourse.mybir as mybir
import concourse.tile as tile
from concourse import bacc
from concourse.bass_utils import run_bass_kernel_spmd

B, S, D, F = 8, 2048, 1024, 64
NCORES = 8
BF = mybir.dt.bfloat16
F32 = mybir.dt.float32
SCALE = 1.0 / float(np.sqrt(np.float32(S)))  # reference scales by sqrt(S)
MASK_BIAS = -30000.0  # exp(x + MASK_BIAS) == 0.0 in f32 for |x| < 1
SC = 512  # matmul moving free-dim
KCH = S // 128  # 16 key chunks
DCH = D // 128  # 8 contraction chunks


def _emit(nc):
    seqT_d = nc.declare_dram_parameter("seqT", [D, S], BF, isOutput=False)
    wq_d = nc.declare_dram_parameter("wq", [D, F], BF, isOutput=False)
    wk_d = nc.declare_dram_parameter("wk", [D, F], BF, isOutput=False)
    wv_d = nc.declare_dram_parameter("wv", [D, F], BF, isOutput=False)
    # misc f32 [128, 19]: col0 rows0:64 = bq, col1 rows0:64 = bk,
    # col2 rows0:64 = bv, cols 3:19 = mask bias per key chunk [128, 16]
    misc_d = nc.declare_dram_parameter("misc", [128, 3 + KCH], F32, isOutput=False)
    identb_d = nc.declare_dram_parameter("identb", [128, 128], BF, isOutput=False)
    identf_d = nc.declare_dram_parameter("identf", [128, 128], F32, isOutput=False)
    out_d = nc.declare_dram_parameter("out", [S, F], F32, isOutput=True)

    with tile.TileContext(nc) as tc:
        _body(nc, tc, seqT_d, wq_d, wk_d, wv_d, misc_d, identb_d, identf_d, out_d)
    nc.compile()


def _body(nc, tc, seqT_d, wq_d, wk_d, wv_d, misc_d, identb_d, identf_d, out_d):
    from contextlib import ExitStack

    with ExitStack() as ctx:
        const = ctx.enter_context(tc.tile_pool(name="const", bufs=1))
        big = ctx.enter_context(tc.tile_pool(name="big", bufs=1))
        sbw = ctx.enter_context(tc.tile_pool(name="sbw", bufs=1))

        # ---- constant loads ----
        wq_sb = const.tile([128, DCH, F], BF, name="wq_sb")
        wk_sb = const.tile([128, DCH, F], BF, name="wk_sb")
        wv_sb = const.tile([128, DCH, F], BF, name="wv_sb")
        nc.sync.dma_start(out=wq_sb[:], in_=wq_d.ap().rearrange("(c p) f -> p c f", p=128))
        nc.sync.dma_start(out=wk_sb[:], in_=wk_d.ap().rearrange("(c p) f -> p c f", p=128))
        nc.sync.dma_start(out=wv_sb[:], in_=wv_d.ap().rearrange("(c p) f -> p c f", p=128))
        misc_sb = const.tile([128, 3 + KCH], F32, name="misc_sb")
        nc.sync.dma_start(out=misc_sb[:], in_=misc_d.ap())
        identb_sb = const.tile([128, 128], BF, name="identb_sb")
        nc.sync.dma_start(out=identb_sb[:], in_=identb_d.ap())
        identf_sb = const.tile([128, 128], F32, name="identf_sb")
        nc.sync.dma_start(out=identf_sb[:], in_=identf_d.ap())

        seqT_sb = big.tile([128, DCH, S], BF, name="seqT_sb")
        for c in range(DCH):
            nc.sync.dma_start(
                out=seqT_sb[:, c, :], in_=seqT_d[c * 128 : (c + 1) * 128, :]
            )

        # preload the exp table set early so the table-load DMA overlaps phase 1
        dummy_sb = const.tile([1, 1], F32, name="dummy_sb")
        nc.scalar.activation(
            out=dummy_sb[:],
            in_=misc_sb[0:1, 0:1],
            func=mybir.ActivationFunctionType.Exp,
            scale=1.0,
        )

        qT_sb = big.tile([F, S], BF, name="qT_sb")
        kT_sb = big.tile([F, S], BF, name="kT_sb")
        vT_sb = big.tile([F, S], BF, name="vT_sb")
        v_sb = big.tile([128, KCH, F + 1], BF, name="v_sb")
        nc.vector.memset(v_sb[:, :, F], 1.0)

        bq_ap = misc_sb[0:F, 0:1]
        bk_ap = misc_sb[0:F, 1:2]
        bv_ap = misc_sb[0:F, 2:3]

        # ---- phase 1: QKV projections (qT/kT/vT = W.T @ seqT) ----
        with tc.tile_pool(name="psA", space="PSUM", bufs=1) as psA:
            for sj in range(S // SC):
                ps_q = psA.tile([F, SC], F32, tag="psq", bufs=2, name=f"ps_q{sj}")
                ps_k = psA.tile([F, SC], F32, tag="psk", bufs=2, name=f"ps_k{sj}")
                ps_v = psA.tile([F, SC], F32, tag="psv", bufs=2, name=f"ps_v{sj}")
                for c in range(DCH):
                    rhs = seqT_sb[:, c, sj * SC : (sj + 1) * SC]
                    st = dict(start=(c == 0), stop=(c == DCH - 1))
                    nc.tensor.matmul(ps_q[:], wq_sb[:, c, :], rhs, **st)
                    nc.tensor.matmul(ps_k[:], wk_sb[:, c, :], rhs, **st)
                    nc.tensor.matmul(ps_v[:], wv_sb[:, c, :], rhs, **st)
                sl = slice(sj * SC, (sj + 1) * SC)
                nc.vector.tensor_scalar_add(qT_sb[:, sl], ps_q[:], bq_ap)
                nc.vector.tensor_scalar_add(kT_sb[:, sl], ps_k[:], bk_ap)
                nc.vector.tensor_scalar_add(vT_sb[:, sl], ps_v[:], bv_ap)

            # ---- phase 1.5: transpose vT back to natural [k, f] layout ----
            for t in range(KCH):
                vtp = psA.tile([128, F], BF, tag="vtp", bufs=2, name=f"vtp{t}")
                nc.tensor.transpose(
                    vtp[:],
                    vT_sb[:, t * 128 : (t + 1) * 128],
                    identb_sb[0:F, 0:F],
                )
                nc.vector.tensor_copy(v_sb[:, t, 0:F], vtp[:])

        # ---- phase 2: scoresT -> exp -> context accumulation ----
        with tc.tile_pool(name="psB", space="PSUM", bufs=1) as psB:
            ctx_ps = [
                psB.tile([F + 1, SC], F32, tag=f"ctx{qc}", bufs=1, name=f"ctx_ps{qc}")
                for qc in range(S // SC)
            ]
            for k in range(KCH):
                ksl = slice(k * 128, (k + 1) * 128)
                for qh in range(2):
                    ps_s = psB.tile(
                        [128, 2 * SC], F32, tag="ps_s", bufs=2, name=f"ps_s_{k}_{qh}"
                    )
                    for qq in range(2):
                        q0 = qh * 2 * SC + qq * SC
                        nc.tensor.matmul(
                            ps_s[:, qq * SC : (qq + 1) * SC],
                            kT_sb[:, ksl],
                            qT_sb[:, q0 : q0 + SC],
                            start=True,
                            stop=True,
                        )
                    expq = sbw.tile(
                        [128, 2 * SC], BF, tag="expq", bufs=3, name=f"expq_{k}_{qh}"
                    )
                    nc.scalar.activation(
                        out=expq[:],
                        in_=ps_s[:],
                        func=mybir.ActivationFunctionType.Exp,
                        bias=misc_sb[:, 3 + k : 4 + k],
                        scale=SCALE,
                    )
                    for qq in range(2):
                        qc = qh * 2 + qq
                        nc.tensor.matmul(
                            ctx_ps[qc][:],
                            v_sb[:, k, :],
                            expq[:, qq * SC : (qq + 1) * SC],
                            start=(k == 0),
                            stop=(k == KCH - 1),
                        )

            ctxT_sb = big.tile([F + 1, S], F32, name="ctxT_sb")
            for qc in range(S // SC):
                nc.vector.tensor_copy(
                    ctxT_sb[:, qc * SC : (qc + 1) * SC], ctx_ps[qc][:]
                )

        # ---- phase 3: transpose ctxT, normalize by the ones-row sums ----
        with tc.tile_pool(name="psC", space="PSUM", bufs=1) as psC:
            out_sb = big.tile([128, S // 128, F], F32, name="out_sb")
            for t in range(S // 128):
                ctp = psC.tile([128, F + 1], F32, tag="ctp", bufs=2, name=f"ctp{t}")
                nc.tensor.transpose(
                    ctp[:],
                    ctxT_sb[:, t * 128 : (t + 1) * 128],
                    identf_sb[0 : F + 1, 0 : F + 1],
                )
                rec = sbw.tile([128, 1], F32, tag="rec", bufs=2, name=f"rec{t}")
                nc.vector.reciprocal(rec[:], ctp[:, F : F + 1])
                nc.vector.tensor_scalar_mul(out_sb[:, t, :], ctp[:, 0:F], rec[:])
            nc.sync.dma_start(
                out=out_d.ap().rearrange("(c p) f -> p c f", p=128), in_=out_sb[:]
            )


_NC_CACHE = None


def _get_nc():
    global _NC_CACHE
    if _NC_CACHE is None:
        nc = bacc.Bacc("TRN2", target_bir_lowering=False, debug=False)
        _emit(nc)
        _NC_CACHE = nc
    return _NC_CACHE


def make_in_maps(seq, mask, Wq, bq, Wk, bk, Wv, bv):
    bf16 = ml_dtypes.bfloat16
    seq = np.asarray(seq, dtype=np.float32)
    mask = np.asarray(mask).astype(bool)
    wq_h = np.ascontiguousarray(np.asarray(Wq, dtype=np.float32)).astype(bf16)
    wk_h = np.ascontiguousarray(np.asarray(Wk, dtype=np.float32)).astype(bf16)
    wv_h = np.ascontiguousarray(np.asarray(Wv, dtype=np.float32)).astype(bf16)
    identb = np.eye(128, dtype=bf16)
    identf = np.eye(128, dtype=np.float32)
    in_maps = []
    for b in range(NCORES):
        seqT = np.ascontiguousarray(seq[b].T).astype(bf16)  # [D, S]
        misc = np.zeros((128, 3 + KCH), dtype=np.float32)
        misc[0:F, 0] = np.asarray(bq, dtype=np.float32)
        misc[0:F, 1] = np.asarray(bk, dtype=np.float32)
        misc[0:F, 2] = np.asarray(bv, dtype=np.float32)
        # mask bias: misc[p, 3+c] applies to key index c*128 + p
        misc[:, 3:] = np.where(mask[b], np.float32(MASK_BIAS), np.float32(0.0)).reshape(
            KCH, 128
        ).T
        in_maps.append(
            {
                "seqT": seqT,
                "wq": wq_h,
                "wk": wk_h,
                "wv": wv_h,
                "misc": misc,
                "identb": identb,
                "identf": identf,
            }
        )
    return in_maps


def run(in_maps, trace=False, **kw):
    nc = _get_nc()
    return run_bass_kernel_spmd(
        nc, in_maps, core_ids=list(range(NCORES)), trace=trace, **kw
    )


def kernel(seq, mask, Wq, bq, Wk, bk, Wv, bv):
    in_maps = make_in_maps(seq, mask, Wq, bq, Wk, bk, Wv, bv)
    res = run(in_maps)
    out = np.stack(
        [np.asarray(res.results[i]["out"], dtype=np.float32) for i in range(NCORES)],
        axis=0,
    )
    return out


# revision 2
# speedup vs baseline: 1.0458x; 1.0458x over previous
"""Single attention head (B=8, S=2048, D_IN=1024, D_OUT=64) on 8 TRN2 NeuronCores.

Strategy: pure data-parallel over batch -- core b computes batch element b's
full attention head. No collectives.

Per-core dataflow (all matmul compute in bf16, f32 PSUM accumulation):
  phase 1: K/Q projections run column-packed on the PE (K -> array cols 0:64 ->
           psum partitions 0:64, Q -> cols 64:128 -> partitions 64:128), V
           separate; the per-feature bias is added during the PSUM->SBUF drain
           on VectorE.  qT is then shifted back to partitions 0:64 with one
           SBUF->SBUF DMA.  vT is re-transposed on the TensorEngine into
           natural [k, 65] layout with an appended ones-column (so the context
           matmul also produces softmax denominators for free).
  phase 2: for each key-chunk [128 keys]:
             scoresT[k, q] = kT_chunk.T @ qT            (PE, K=64)
             e = exp(scores/sqrt(S) + mask_bias[k])     (ScalarE; mask rides
                 the per-partition bias port; no max-subtraction needed since
                 |scores/sqrt(S)| << 1 and masked lanes underflow to 0)
             ctxT[0:64, q] += v_chunk.T @ e; ctxT[64, q] += sum_k e  (PE)
  phase 3: transpose ctxT back to [q, 65], multiply rows by 1/ctx[.., 64],
           DMA the [2048, 64] f32 result out.
"""

import numpy as np
import ml_dtypes

import concourse.bass as bass  # noqa: F401  (bass types used via tile/bacc)
import concourse.mybir as mybir
import concourse.tile as tile
from concourse import bacc
from concourse.bass_utils import run_bass_kernel_spmd

B, S, D, F = 8, 2048, 1024, 64
NCORES = 8
BF = mybir.dt.bfloat16
F32 = mybir.dt.float32
SCALE = 1.0 / float(np.sqrt(np.float32(S)))  # reference scales by sqrt(S)
MASK_BIAS = -30000.0  # exp(x + MASK_BIAS) == 0.0 in f32 for |x| < 1
SC = 512  # matmul moving free-dim
KCH = S // 128  # 16 key chunks
DCH = D // 128  # 8 contraction chunks


def _emit(nc):
    seqT_d = nc.declare_dram_parameter("seqT", [D, S], BF, isOutput=False)
    wq_d = nc.declare_dram_parameter("wq", [D, F], BF, isOutput=False)
    wk_d = nc.declare_dram_parameter("wk", [D, F], BF, isOutput=False)
    wv_d = nc.declare_dram_parameter("wv", [D, F], BF, isOutput=False)
    # misc f32 [128, 19]: col0 rows64:128 = bq, col1 rows0:64 = bk,
    # col2 rows0:64 = bv, cols 3:19 = mask bias per key chunk [128, 16]
    misc_d = nc.declare_dram_parameter("misc", [128, 3 + KCH], F32, isOutput=False)
    identb_d = nc.declare_dram_parameter("identb", [128, 128], BF, isOutput=False)
    identf_d = nc.declare_dram_parameter("identf", [128, 128], F32, isOutput=False)
    out_d = nc.declare_dram_parameter("out", [S, F], F32, isOutput=True)

    with tile.TileContext(nc) as tc:
        _body(nc, tc, seqT_d, wq_d, wk_d, wv_d, misc_d, identb_d, identf_d, out_d)
    nc.compile()


def _body(nc, tc, seqT_d, wq_d, wk_d, wv_d, misc_d, identb_d, identf_d, out_d):
    from contextlib import ExitStack

    with ExitStack() as ctx:
        const = ctx.enter_context(tc.tile_pool(name="const", bufs=1))
        big = ctx.enter_context(tc.tile_pool(name="big", bufs=1))
        sbw = ctx.enter_context(tc.tile_pool(name="sbw", bufs=1))

        # ---- constant loads (scalar-engine HWDGE queue) ----
        misc_sb = const.tile([128, 3 + KCH], F32, name="misc_sb")
        nc.scalar.dma_start(out=misc_sb[:], in_=misc_d.ap())
        wq_sb = const.tile([128, DCH, F], BF, name="wq_sb")
        wk_sb = const.tile([128, DCH, F], BF, name="wk_sb")
        wv_sb = const.tile([128, DCH, F], BF, name="wv_sb")
        nc.scalar.dma_start(out=wq_sb[:], in_=wq_d.ap().rearrange("(c p) f -> p c f", p=128))
        nc.scalar.dma_start(out=wk_sb[:], in_=wk_d.ap().rearrange("(c p) f -> p c f", p=128))
        nc.scalar.dma_start(out=wv_sb[:], in_=wv_d.ap().rearrange("(c p) f -> p c f", p=128))
        identb_sb = const.tile([128, 128], BF, name="identb_sb")
        nc.scalar.dma_start(out=identb_sb[:], in_=identb_d.ap())
        identf_sb = const.tile([128, 128], F32, name="identf_sb")
        nc.scalar.dma_start(out=identf_sb[:], in_=identf_d.ap())

        # seq chunks as separate tiles so QKV matmuls start on chunk 0 arrival
        seqc = []
        for c in range(DCH):
            t = big.tile([128, S], BF, name=f"seqc{c}")
            nc.sync.dma_start(out=t[:], in_=seqT_d[c * 128 : (c + 1) * 128, :])
            seqc.append(t)

        # preload the exp table set early so the table-load DMA overlaps phase 1
        dummy_sb = const.tile([1, 1], F32, name="dummy_sb")
        nc.scalar.activation(
            out=dummy_sb[:],
            in_=misc_sb[0:1, 0:1],
            func=mybir.ActivationFunctionType.Exp,
            scale=1.0,
        )

        qT_sb = big.tile([F, S], BF, name="qT_sb")
        qTh_sb = big.tile([128, S], BF, name="qTh_sb")  # q lives on rows 64:128
        kT_sb = big.tile([F, S], BF, name="kT_sb")
        vT_sb = big.tile([F, S], BF, name="vT_sb")
        v_sb = big.tile([128, KCH, F + 1], BF, name="v_sb")
        nc.vector.memset(v_sb[:, :, F], 1.0)

        bq_hi_ap = misc_sb[64:128, 0:1]
        bk_ap = misc_sb[0:F, 1:2]
        bv_ap = misc_sb[0:F, 2:3]

        # ---- phase 1: projections; K col-group 0, Q col-group 1 (concurrent) ----
        with tc.tile_pool(name="psA", space="PSUM", bufs=1) as psA:
            for sj in range(S // SC):
                ps_k = psA.tile([F, SC], F32, tag="psk", bufs=2, name=f"ps_k{sj}")
                ps_qh = psA.tile([128, SC], F32, tag="psq", bufs=2, name=f"ps_qh{sj}")
                ps_v = psA.tile([F, SC], F32, tag="psv", bufs=2, name=f"ps_v{sj}")
                for c in range(DCH):
                    rhs = seqc[c][:, sj * SC : (sj + 1) * SC]
                    st = dict(start=(c == 0), stop=(c == DCH - 1))
                    nc.tensor.matmul(
                        ps_k[:], wk_sb[:, c, :], rhs, tile_position=(0, 0), **st
                    )
                    nc.tensor.matmul(
                        ps_qh[64:128, :], wq_sb[:, c, :], rhs, tile_position=(0, 64), **st
                    )
                    nc.tensor.matmul(ps_v[:], wv_sb[:, c, :], rhs, **st)
                sl = slice(sj * SC, (sj + 1) * SC)
                nc.vector.tensor_scalar_add(kT_sb[:, sl], ps_k[:], bk_ap)
                nc.vector.tensor_scalar_add(qTh_sb[64:128, sl], ps_qh[64:128, :], bq_hi_ap)
                nc.vector.tensor_scalar_add(vT_sb[:, sl], ps_v[:], bv_ap)

            # shift qT down to partitions 0:64 (engines are lane-locked; DMA isn't)
            nc.gpsimd.dma_start(out=qT_sb[:], in_=qTh_sb[64:128, :])

            # ---- phase 1.5: transpose vT back to natural [k, f] layout ----
            for t in range(KCH):
                vtp = psA.tile([128, F], BF, tag="vtp", bufs=2, name=f"vtp{t}")
                nc.tensor.transpose(
                    vtp[:],
                    vT_sb[:, t * 128 : (t + 1) * 128],
                    identb_sb[0:F, 0:F],
                )
                nc.vector.tensor_copy(v_sb[:, t, 0:F], vtp[:])

        # ---- phase 2: scoresT -> exp -> context accumulation ----
        with tc.tile_pool(name="psB", space="PSUM", bufs=1) as psB:
            ctx_ps = [
                psB.tile([F + 1, SC], F32, tag=f"ctx{qc}", bufs=1, name=f"ctx_ps{qc}")
                for qc in range(S // SC)
            ]
            for k in range(KCH):
                ksl = slice(k * 128, (k + 1) * 128)
                for qh in range(2):
                    ps_s = psB.tile(
                        [128, 2 * SC], F32, tag="ps_s", bufs=2, name=f"ps_s_{k}_{qh}"
                    )
                    for qq in range(2):
                        q0 = qh * 2 * SC + qq * SC
                        nc.tensor.matmul(
                            ps_s[:, qq * SC : (qq + 1) * SC],
                            kT_sb[:, ksl],
                            qT_sb[:, q0 : q0 + SC],
                            start=True,
                            stop=True,
                        )
                    expq = sbw.tile(
                        [128, 2 * SC], BF, tag="expq", bufs=3, name=f"expq_{k}_{qh}"
                    )
                    nc.scalar.activation(
                        out=expq[:],
                        in_=ps_s[:],
                        func=mybir.ActivationFunctionType.Exp,
                        bias=misc_sb[:, 3 + k : 4 + k],
                        scale=SCALE,
                    )
                    for qq in range(2):
                        qc = qh * 2 + qq
                        nc.tensor.matmul(
                            ctx_ps[qc][:],
                            v_sb[:, k, :],
                            expq[:, qq * SC : (qq + 1) * SC],
                            start=(k == 0),
                            stop=(k == KCH - 1),
                        )

            ctxT_sb = big.tile([F + 1, S], F32, name="ctxT_sb")
            for qc in range(S // SC):
                nc.vector.tensor_copy(
                    ctxT_sb[:, qc * SC : (qc + 1) * SC], ctx_ps[qc][:]
                )

        # ---- phase 3: transpose ctxT, normalize by the ones-row sums ----
        with tc.tile_pool(name="psC", space="PSUM", bufs=1) as psC:
            out_sb = big.tile([128, S // 128, F], F32, name="out_sb")
            for t in range(S // 128):
                ctp = psC.tile([128, F + 1], F32, tag="ctp", bufs=2, name=f"ctp{t}")
                nc.tensor.transpose(
                    ctp[:],
                    ctxT_sb[:, t * 128 : (t + 1) * 128],
                    identf_sb[0 : F + 1, 0 : F + 1],
                )
                rec = sbw.tile([128, 1], F32, tag="rec", bufs=2, name=f"rec{t}")
                nc.vector.reciprocal(rec[:], ctp[:, F : F + 1])
                nc.vector.tensor_scalar_mul(out_sb[:, t, :], ctp[:, 0:F], rec[:])
            nc.sync.dma_start(
                out=out_d.ap().rearrange("(c p) f -> p c f", p=128), in_=out_sb[:]
            )


_NC_CACHE = None


def _get_nc():
    global _NC_CACHE
    if _NC_CACHE is None:
        nc = bacc.Bacc("TRN2", target_bir_lowering=False, debug=False)
        _emit(nc)
        _NC_CACHE = nc
    return _NC_CACHE


def make_in_maps(seq, mask, Wq, bq, Wk, bk, Wv, bv):
    bf16 = ml_dtypes.bfloat16
    seq = np.asarray(seq, dtype=np.float32)
    mask = np.asarray(mask).astype(bool)
    wq_h = np.ascontiguousarray(np.asarray(Wq, dtype=np.float32)).astype(bf16)
    wk_h = np.ascontiguousarray(np.asarray(Wk, dtype=np.float32)).astype(bf16)
    wv_h = np.ascontiguousarray(np.asarray(Wv, dtype=np.float32)).astype(bf16)
    identb = np.eye(128, dtype=bf16)
    identf = np.eye(128, dtype=np.float32)
    in_maps = []
    for b in range(NCORES):
        seqT = np.ascontiguousarray(seq[b].T).astype(bf16)  # [D, S]
        misc = np.zeros((128, 3 + KCH), dtype=np.float32)
        misc[64:128, 0] = np.asarray(bq, dtype=np.float32)
        misc[0:F, 1] = np.asarray(bk, dtype=np.float32)
        misc[0:F, 2] = np.asarray(bv, dtype=np.float32)
        # mask bias: misc[p, 3+c] applies to key index c*128 + p
        misc[:, 3:] = np.where(mask[b], np.float32(MASK_BIAS), np.float32(0.0)).reshape(
            KCH, 128
        ).T
        in_maps.append(
            {
                "seqT": seqT,
                "wq": wq_h,
                "wk": wk_h,
                "wv": wv_h,
                "misc": misc,
                "identb": identb,
                "identf": identf,
            }
        )
    return in_maps


def run(in_maps, trace=False, **kw):
    nc = _get_nc()
    return run_bass_kernel_spmd(
        nc, in_maps, core_ids=list(range(NCORES)), trace=trace, **kw
    )


def kernel(seq, mask, Wq, bq, Wk, bk, Wv, bv):
    in_maps = make_in_maps(seq, mask, Wq, bq, Wk, bk, Wv, bv)
    res = run(in_maps)
    out = np.stack(
        [np.asarray(res.results[i]["out"], dtype=np.float32) for i in range(NCORES)],
        axis=0,
    )
    return out


# revision 3
# speedup vs baseline: 1.3114x; 1.2539x over previous
"""Single attention head (B=8, S=2048, D_IN=1024, D_OUT=64) on 8 TRN2 NeuronCores.

Strategy: pure data-parallel over batch -- core b computes batch element b's
full attention head. No collectives.

Per-core dataflow (all matmul compute in bf16, f32 PSUM accumulation):
  phase 1: K/Q projections on the PE (K -> psum partitions 0:64 via col group 0,
           Q -> partitions 64:128 via col group 1), V separate; the per-feature
           bias is added during the PSUM->SBUF drain on VectorE.  One
           SBUF->SBUF DMA makes the partition-shifted copies needed below
           (qT on rows 0:64, kT on rows 64:128).  vT is re-transposed on the
           TensorEngine into natural [k, 65] layout; masked-out keys get their
           V rows (and the appended ones-column) ZEROED, which applies the
           attention mask exactly: masked keys then contribute nothing to the
           context sum or to the softmax denominator.
  phase 2: key-chunk PAIRS are row-tiled on the PE (chunk A on array rows
           0:64, chunk B on rows 64:128 -- disjoint row groups, so the two
           K=64 matmuls run concurrently) into one [128, 1024] psum pair-tile;
           ONE exp activation covers the pair; two context matmuls accumulate
           ctxT[65, q] (row 64 of ve = mask01 produces the softmax
           denominators for free).  No max-subtraction is needed:
           |scores/sqrt(S)| << 1.
  phase 3: transpose ctxT back to [q, 65], multiply rows by 1/ctx[.., 64],
           DMA the [2048, 64] f32 result out.
"""

import numpy as np
import ml_dtypes

import concourse.bass as bass  # noqa: F401  (bass types used via tile/bacc)
import concourse.mybir as mybir
import concourse.tile as tile
from concourse import bacc
from concourse.bass_utils import run_bass_kernel_spmd

B, S, D, F = 8, 2048, 1024, 64
NCORES = 8
BF = mybir.dt.bfloat16
F32 = mybir.dt.float32
SCALE = 1.0 / float(np.sqrt(np.float32(S)))  # reference scales by sqrt(S)
SC = 512  # matmul moving free-dim
KCH = S // 128  # 16 key chunks
DCH = D // 128  # 8 contraction chunks


def _emit(nc):
    seqT_d = nc.declare_dram_parameter("seqT", [D, S], BF, isOutput=False)
    wq_d = nc.declare_dram_parameter("wq", [D, F], BF, isOutput=False)
    wk_d = nc.declare_dram_parameter("wk", [D, F], BF, isOutput=False)
    wv_d = nc.declare_dram_parameter("wv", [D, F], BF, isOutput=False)
    # misc f32 [128, 19]: col0 rows64:128 = bq, col1 rows0:64 = bk,
    # col2 rows0:64 = bv, cols 3:19 = 0/1 keep-mask per key chunk [128, 16]
    misc_d = nc.declare_dram_parameter("misc", [128, 3 + KCH], F32, isOutput=False)
    identb_d = nc.declare_dram_parameter("identb", [128, 128], BF, isOutput=False)
    identf_d = nc.declare_dram_parameter("identf", [128, 128], F32, isOutput=False)
    out_d = nc.declare_dram_parameter("out", [S, F], F32, isOutput=True)

    with tile.TileContext(nc) as tc:
        _body(nc, tc, seqT_d, wq_d, wk_d, wv_d, misc_d, identb_d, identf_d, out_d)
    nc.compile()


def _body(nc, tc, seqT_d, wq_d, wk_d, wv_d, misc_d, identb_d, identf_d, out_d):
    from contextlib import ExitStack

    with ExitStack() as ctx:
        const = ctx.enter_context(tc.tile_pool(name="const", bufs=1))
        big = ctx.enter_context(tc.tile_pool(name="big", bufs=1))
        sbw = ctx.enter_context(tc.tile_pool(name="sbw", bufs=1))

        # ---- constant loads (scalar-engine HWDGE queue) ----
        misc_sb = const.tile([128, 3 + KCH], F32, name="misc_sb")
        nc.scalar.dma_start(out=misc_sb[:], in_=misc_d.ap())
        wq_sb = const.tile([128, DCH, F], BF, name="wq_sb")
        wk_sb = const.tile([128, DCH, F], BF, name="wk_sb")
        wv_sb = const.tile([128, DCH, F], BF, name="wv_sb")
        nc.scalar.dma_start(out=wq_sb[:], in_=wq_d.ap().rearrange("(c p) f -> p c f", p=128))
        nc.scalar.dma_start(out=wk_sb[:], in_=wk_d.ap().rearrange("(c p) f -> p c f", p=128))
        nc.scalar.dma_start(out=wv_sb[:], in_=wv_d.ap().rearrange("(c p) f -> p c f", p=128))
        identb_sb = const.tile([128, 128], BF, name="identb_sb")
        nc.scalar.dma_start(out=identb_sb[:], in_=identb_d.ap())
        identf_sb = const.tile([128, 128], F32, name="identf_sb")
        nc.scalar.dma_start(out=identf_sb[:], in_=identf_d.ap())

        # seq chunks as separate tiles so QKV matmuls start on chunk 0 arrival
        seqc = []
        for c in range(DCH):
            t = big.tile([128, S], BF, name=f"seqc{c}")
            nc.sync.dma_start(out=t[:], in_=seqT_d[c * 128 : (c + 1) * 128, :])
            seqc.append(t)

        # preload the exp table set early so the table-load DMA overlaps phase 1
        dummy_sb = const.tile([1, 1], F32, name="dummy_sb")
        nc.scalar.activation(
            out=dummy_sb[:],
            in_=misc_sb[0:1, 0:1],
            func=mybir.ActivationFunctionType.Exp,
            scale=1.0,
        )

        qT_sb = big.tile([F, S], BF, name="qT_sb")  # q on rows 0:64 (pair A rhs)
        qTh_sb = big.tile([128, S], BF, name="qTh_sb")  # q on rows 64:128 (pair B rhs)
        kT_sb = big.tile([F, S], BF, name="kT_sb")  # k on rows 0:64 (pair A lhsT)
        kTh_sb = big.tile([128, S], BF, name="kTh_sb")  # k on rows 64:128 (pair B lhsT)
        vT_sb = big.tile([F, S], BF, name="vT_sb")
        v_sb = big.tile([128, KCH, F + 1], BF, name="v_sb")

        bq_hi_ap = misc_sb[64:128, 0:1]
        bk_ap = misc_sb[0:F, 1:2]
        bv_ap = misc_sb[0:F, 2:3]
        mask01 = misc_sb[:, 3:]  # [128, 16] 1.0 = keep, 0.0 = masked out

        # ones-column of ve := keep-mask (masked keys contribute 0 to the sums)
        nc.vector.tensor_copy(v_sb[:, :, F], mask01)

        # ---- phase 1: projections; K col-group 0, Q col-group 1 ----
        with tc.tile_pool(name="psA", space="PSUM", bufs=1) as psA:
            for sj in range(S // SC):
                ps_k = psA.tile([F, SC], F32, tag="psk", bufs=2, name=f"ps_k{sj}")
                ps_qh = psA.tile([128, SC], F32, tag="psq", bufs=2, name=f"ps_qh{sj}")
                ps_v = psA.tile([F, SC], F32, tag="psv", bufs=2, name=f"ps_v{sj}")
                for c in range(DCH):
                    rhs = seqc[c][:, sj * SC : (sj + 1) * SC]
                    st = dict(start=(c == 0), stop=(c == DCH - 1))
                    nc.tensor.matmul(
                        ps_k[:], wk_sb[:, c, :], rhs, tile_position=(0, 0), **st
                    )
                    nc.tensor.matmul(
                        ps_qh[64:128, :], wq_sb[:, c, :], rhs, tile_position=(0, 64), **st
                    )
                    nc.tensor.matmul(ps_v[:], wv_sb[:, c, :], rhs, **st)
                sl = slice(sj * SC, (sj + 1) * SC)
                nc.vector.tensor_scalar_add(kT_sb[:, sl], ps_k[:], bk_ap)
                nc.vector.tensor_scalar_add(qTh_sb[64:128, sl], ps_qh[64:128, :], bq_hi_ap)
                nc.vector.tensor_scalar_add(vT_sb[:, sl], ps_v[:], bv_ap)

                # transpose this sj's vT chunks into natural [k, f] layout,
                # zeroing masked keys' V rows (applies the attention mask)
                for t in range(sj * 4, sj * 4 + 4):
                    vtp = psA.tile([128, F], BF, tag="vtp", bufs=2, name=f"vtp{t}")
                    nc.tensor.transpose(
                        vtp[:],
                        vT_sb[:, t * 128 : (t + 1) * 128],
                        identb_sb[0:F, 0:F],
                    )
                    nc.vector.tensor_scalar_mul(
                        v_sb[:, t, 0:F], vtp[:], mask01[:, t : t + 1]
                    )

            # partition-shifted copies (engines are lane-locked; DMA isn't)
            nc.gpsimd.dma_start(out=qT_sb[:], in_=qTh_sb[64:128, :])
            nc.gpsimd.dma_start(out=kTh_sb[64:128, :], in_=kT_sb[:])

        # ---- phase 2: paired scoresT -> exp -> context accumulation ----
        with tc.tile_pool(name="psB", space="PSUM", bufs=1) as psB:
            ctx_ps = [
                psB.tile([F + 1, SC], F32, tag=f"ctx{qc}", bufs=1, name=f"ctx_ps{qc}")
                for qc in range(S // SC)
            ]
            for p in range(KCH // 2):
                ka, kb = 2 * p, 2 * p + 1
                ksa = slice(ka * 128, (ka + 1) * 128)
                ksb = slice(kb * 128, (kb + 1) * 128)
                for qc in range(S // SC):
                    qsl = slice(qc * SC, (qc + 1) * SC)
                    ps_pair = psB.tile(
                        [128, 2 * SC], F32, tag="pspair", bufs=2, name=f"ps_pair_{p}_{qc}"
                    )
                    # chunk A on array rows 0:64, chunk B on rows 64:128 --
                    # disjoint row groups run concurrently on the PE
                    nc.tensor.matmul(
                        ps_pair[:, 0:SC],
                        kT_sb[:, ksa],
                        qT_sb[:, qsl],
                        start=True,
                        stop=True,
                    )
                    nc.tensor.matmul(
                        ps_pair[:, SC : 2 * SC],
                        kTh_sb[64:128, ksb],
                        qTh_sb[64:128, qsl],
                        start=True,
                        stop=True,
                    )
                    expq = sbw.tile(
                        [128, 2 * SC], BF, tag="expq", bufs=3, name=f"expq_{p}_{qc}"
                    )
                    nc.scalar.activation(
                        out=expq[:],
                        in_=ps_pair[:],
                        func=mybir.ActivationFunctionType.Exp,
                        scale=SCALE,
                    )
                    nc.tensor.matmul(
                        ctx_ps[qc][:],
                        v_sb[:, ka, :],
                        expq[:, 0:SC],
                        start=(p == 0),
                        stop=False,
                    )
                    nc.tensor.matmul(
                        ctx_ps[qc][:],
                        v_sb[:, kb, :],
                        expq[:, SC : 2 * SC],
                        start=False,
                        stop=(p == KCH // 2 - 1),
                    )

            ctxT_sb = big.tile([F + 1, S], F32, name="ctxT_sb")
            for qc in range(S // SC):
                nc.vector.tensor_copy(
                    ctxT_sb[:, qc * SC : (qc + 1) * SC], ctx_ps[qc][:]
                )

        # ---- phase 3: transpose ctxT, normalize by the ones-row sums ----
        with tc.tile_pool(name="psC", space="PSUM", bufs=1) as psC:
            out_sb = big.tile([128, S // 128, F], F32, name="out_sb")
            for t in range(S // 128):
                ctp = psC.tile([128, F + 1], F32, tag="ctp", bufs=2, name=f"ctp{t}")
                nc.tensor.transpose(
                    ctp[:],
                    ctxT_sb[:, t * 128 : (t + 1) * 128],
                    identf_sb[0 : F + 1, 0 : F + 1],
                )
                rec = sbw.tile([128, 1], F32, tag="rec", bufs=2, name=f"rec{t}")
                nc.vector.reciprocal(rec[:], ctp[:, F : F + 1])
                nc.vector.tensor_scalar_mul(out_sb[:, t, :], ctp[:, 0:F], rec[:])
            nc.sync.dma_start(
                out=out_d.ap().rearrange("(c p) f -> p c f", p=128), in_=out_sb[:]
            )


_NC_CACHE = None


def _get_nc():
    global _NC_CACHE
    if _NC_CACHE is None:
        nc = bacc.Bacc("TRN2", target_bir_lowering=False, debug=False)
        _emit(nc)
        _NC_CACHE = nc
    return _NC_CACHE


def make_in_maps(seq, mask, Wq, bq, Wk, bk, Wv, bv):
    bf16 = ml_dtypes.bfloat16
    seq = np.asarray(seq, dtype=np.float32)
    mask = np.asarray(mask).astype(bool)
    wq_h = np.ascontiguousarray(np.asarray(Wq, dtype=np.float32)).astype(bf16)
    wk_h = np.ascontiguousarray(np.asarray(Wk, dtype=np.float32)).astype(bf16)
    wv_h = np.ascontiguousarray(np.asarray(Wv, dtype=np.float32)).astype(bf16)
    identb = np.eye(128, dtype=bf16)
    identf = np.eye(128, dtype=np.float32)
    in_maps = []
    for b in range(NCORES):
        seqT = np.ascontiguousarray(seq[b].T).astype(bf16)  # [D, S]
        misc = np.zeros((128, 3 + KCH), dtype=np.float32)
        misc[64:128, 0] = np.asarray(bq, dtype=np.float32)
        misc[0:F, 1] = np.asarray(bk, dtype=np.float32)
        misc[0:F, 2] = np.asarray(bv, dtype=np.float32)
        # keep-mask: misc[p, 3+c] = 0.0 if key c*128+p is masked out else 1.0
        misc[:, 3:] = np.where(mask[b], np.float32(0.0), np.float32(1.0)).reshape(
            KCH, 128
        ).T
        in_maps.append(
            {
                "seqT": seqT,
                "wq": wq_h,
                "wk": wk_h,
                "wv": wv_h,
                "misc": misc,
                "identb": identb,
                "identf": identf,
            }
        )
    return in_maps


def run(in_maps, trace=False, **kw):
    nc = _get_nc()
    return run_bass_kernel_spmd(
        nc, in_maps, core_ids=list(range(NCORES)), trace=trace, **kw
    )


def kernel(seq, mask, Wq, bq, Wk, bk, Wv, bv):
    in_maps = make_in_maps(seq, mask, Wq, bq, Wk, bk, Wv, bv)
    res = run(in_maps)
    out = np.stack(
        [np.asarray(res.results[i]["out"], dtype=np.float32) for i in range(NCORES)],
        axis=0,
    )
    return out
